# revision 1
# baseline (speedup 1.0000x reference)
"""Multi-head attention (B=4, L=2048, D=1024, H=16) on 8 Trainium2 NeuronCores.

Sharding: core c = (batch b = c//2, query-half qh = c%2). Each core computes
all 16 heads for its 1024 query rows against the full 2048 keys/values of its
batch. Fully SPMD, no collectives. Host does transpose/slice prep and
concatenation gather only.

Per-core pipeline (all matmuls fp32r, N=512):
  1. kpT[dh,1024 x L,2048], qpT[dh,1024 x Lq,1024] = W^T-major projections
     vp[L,2048 x dh,1024]  = value projection (L-major)
  2. per head-pair, per 512-col query chunk:
       S^T[j,i] = kpT^T qpT (row-packed pairs, K=64)
       P = exp(S^T) (no max subtraction; scores ~N(0,1), |s| < ~7)
       outT[dh,i] += vp^T P (col-packed pairs), Z[i] += 1^T P (ones-matmul)
       outT /= Z (reciprocal + K=1 broadcast matmul + DVE mul)
  3. out[l,o] = outT^T woT + bo  (streamed from DRAM scratch)
"""

import sys

if "/opt/trn_rl_repo" not in sys.path:
    sys.path.insert(0, "/opt/trn_rl_repo")

import numpy as np

import concourse.bacc as bacc
import concourse.tile as tile
from concourse import mybir
from concourse.bass_utils import run_bass_kernel_spmd

N_CORES = 8
B, L, D = 4, 2048, 1024
NH, DH = 16, 64          # heads, head dim
LQ = L // 2              # query rows per core
F32 = mybir.dt.float32
F32R = mybir.dt.float32r

KC = D // 128            # 8 contraction chunks for projections
MQ = LQ // 128           # 8 l-chunks per core
NJ = L // 128            # 16 key j-chunks
NI = LQ // 512           # 2 query i-chunks of 512
NPAIR = NH // 2          # 8 head pairs
JGROUPS = [3, 3, 3, 3, 3, 1]  # 16 j-chunks split into exp groups


def build_program(dbg=False):
    nc = bacc.Bacc("TRN2", target_bir_lowering=False, debug=False,
                   num_devices=N_CORES)
    with tile.TileContext(nc) as tc:
        _emit(nc, tc, dbg=dbg)
    nc.compile()
    return nc


def _emit(nc, tc, dbg=False):
    from contextlib import ExitStack

    top = ExitStack()
    dram = top.enter_context(tc.tile_pool(name="dram", bufs=1, space="DRAM"))
    xqT = dram.tile([D, LQ], F32R, kind="ExternalInput", name="xqT", uniquify=False)
    xkT = dram.tile([D, L], F32R, kind="ExternalInput", name="xkT", uniquify=False)
    xvT = dram.tile([D, L], F32R, kind="ExternalInput", name="xvT", uniquify=False)
    wqT = dram.tile([D, D], F32R, kind="ExternalInput", name="wqT", uniquify=False)
    wkT = dram.tile([D, D], F32R, kind="ExternalInput", name="wkT", uniquify=False)
    wvT = dram.tile([D, D], F32R, kind="ExternalInput", name="wvT", uniquify=False)
    woT = dram.tile([D, D], F32R, kind="ExternalInput", name="woT", uniquify=False)
    bqc = dram.tile([128, KC], F32, kind="ExternalInput", name="bqc", uniquify=False)
    bkc = dram.tile([128, KC], F32, kind="ExternalInput", name="bkc", uniquify=False)
    bvr = dram.tile([1, D], F32R, kind="ExternalInput", name="bvr", uniquify=False)
    bor = dram.tile([1, D], F32R, kind="ExternalInput", name="bor", uniquify=False)
    c_or = dram.tile([1, 128], F32R, kind="ExternalInput", name="c_or", uniquify=False)
    c_oc = dram.tile([128, 1], F32R, kind="ExternalInput", name="c_oc", uniquify=False)
    c_sel = dram.tile([2, 128], F32R, kind="ExternalInput", name="c_sel", uniquify=False)
    c_vone = dram.tile([128, NH], F32R, kind="ExternalInput", name="c_vone",
                       uniquify=False)
    out = dram.tile([LQ, D], F32, kind="ExternalOutput", name="out", uniquify=False)
    outT_d = dram.tile([D, LQ], F32R, kind="Internal", name="outT_d")
    if dbg:
        qpT_dbg = dram.tile([D, LQ], F32R, kind="ExternalOutput",
                            name="qpT_dbg", uniquify=False)
        kpT_dbg = dram.tile([D, L], F32R, kind="ExternalOutput",
                            name="kpT_dbg", uniquify=False)
        vpa_dbg = dram.tile([NJ * 128, NH * 65], F32R, kind="ExternalOutput",
                            name="vpa_dbg", uniquify=False)
        outT_dbg = dram.tile([D, LQ], F32R, kind="ExternalOutput",
                             name="outT_dbg", uniquify=False)
        e_dbg = dram.tile([128, 1536], F32R, kind="ExternalOutput",
                          name="e_dbg", uniquify=False)
        z_dbg = dram.tile([2, 512], F32, kind="ExternalOutput",
                          name="z_dbg", uniquify=False)

    # persistent SBUF
    pers = top.enter_context(tc.tile_pool(name="pers", bufs=1))
    kpT = [pers.tile([128, L], F32R, name=f"kpT{m}") for m in range(KC)]
    qpT = [pers.tile([128, LQ], F32R, name=f"qpT{m}") for m in range(KC)]
    # vpa: per j-chunk, 16 heads x (64 value cols + 1 ones col) -> Z via PV
    vpa = [pers.tile([128, NH * 65], F32R, name=f"vpa{m}") for m in range(NJ)]
    ones1 = pers.tile([1, 128], F32R, name="ones1")
    ones128 = pers.tile([128, 1], F32R, name="ones128")
    sel2 = pers.tile([2, 128], F32R, name="sel2")
    bq_sb = pers.tile([128, KC], F32, name="bq_sb")
    bk_sb = pers.tile([128, KC], F32, name="bk_sb")
    bv_sb = pers.tile([1, D], F32R, name="bv_sb")
    bo_sb = pers.tile([1, D], F32R, name="bo_sb")

    nc.sync.dma_start(out=ones1[:], in_=c_or[:])
    nc.sync.dma_start(out=ones128[:], in_=c_oc[:])
    nc.sync.dma_start(out=sel2[:], in_=c_sel[:])
    for m in range(NJ):
        nc.sync.dma_start(
            out=vpa[m].rearrange("p (h c) -> p h c", c=65)[:, :, 64:65],
            in_=c_vone.rearrange("p (h o) -> p h o", o=1))
    nc.sync.dma_start(out=bq_sb[:], in_=bqc[:])
    nc.sync.dma_start(out=bk_sb[:], in_=bkc[:])
    nc.sync.dma_start(out=bv_sb[:], in_=bvr[:])
    nc.sync.dma_start(out=bo_sb[:], in_=bor[:])

    xkT_r = xkT.rearrange("(kc p) l -> p kc l", p=128)
    xqT_r = xqT.rearrange("(kc p) l -> p kc l", p=128)
    xvT_r = xvT.rearrange("(kc p) l -> p kc l", p=128)
    wqT_r = wqT.rearrange("(kc p) m -> p kc m", p=128)
    wkT_r = wkT.rearrange("(kc p) m -> p kc m", p=128)
    wvT_r = wvT.rearrange("(kc p) m -> p kc m", p=128)
    woT_r = woT.rearrange("(kc p) m -> p kc m", p=128)
    outT_r = outT_d.rearrange("(kc p) l -> p kc l", p=128)

    # ---- phase 1: projections -------------------------------------------
    with ExitStack() as proj_ctx:
        px = proj_ctx.enter_context(tc.tile_pool(name="px", bufs=2))
        pw = proj_ctx.enter_context(tc.tile_pool(name="pw", bufs=3))
        pp = proj_ctx.enter_context(tc.tile_pool(name="pp", bufs=4, space="PSUM"))

        # kpT / qpT:  psum[dh128, l512] = sum_kc wT[:,kc,m128].T @ xT[:,kc,n512]
        # x streamed in 256-wide half-blocks to fit SBUF (fp32r full-rate >=256)
        for (w_r, x_r, dst, bias, nn) in (
            (wkT_r, xkT_r, kpT, bk_sb, L // 256),
            (wqT_r, xqT_r, qpT, bq_sb, LQ // 256),
        ):
            for n in range(0, nn, 2):
                xa = px.tile([128, KC, 256], F32R, tag="px")
                xb = px.tile([128, KC, 256], F32R, tag="px")
                nc.sync.dma_start(out=xa[:], in_=x_r[:, :, n * 256:(n + 1) * 256])
                nc.sync.dma_start(out=xb[:], in_=x_r[:, :, (n + 1) * 256:(n + 2) * 256])
                for m in range(KC):
                    wb = pw.tile([128, KC, 128], F32R, tag="pw")
                    nc.sync.dma_start(out=wb[:], in_=w_r[:, :, m * 128:(m + 1) * 128])
                    ps = pp.tile([128, 512], F32, tag="pp")
                    # fp32r accumulation groups must not interleave within a
                    # psum bank: run the two 256-wide halves sequentially
                    for kc in range(KC):
                        nc.tensor.matmul(ps[:, 0:256], wb[:, kc, :], xa[:, kc, :],
                                         start=(kc == 0), stop=(kc == KC - 1))
                    for kc in range(KC):
                        nc.tensor.matmul(ps[:, 256:512], wb[:, kc, :], xb[:, kc, :],
                                         start=(kc == 0), stop=(kc == KC - 1))
                    nc.vector.tensor_scalar_add(
                        dst[m][:, n * 256:(n + 2) * 256], ps[:], bias[:, m:m + 1])

        # vp: psum[l128, dh512] = bias + sum_kc xvT[:,kc,m128].T @ wvT[:,kc,n512]
        for n in range(D // 512):
            wa = px.tile([128, KC, 256], F32R, tag="px")
            wb2 = px.tile([128, KC, 256], F32R, tag="px")
            nc.sync.dma_start(out=wa[:], in_=wvT_r[:, :, n * 512:n * 512 + 256])
            nc.sync.dma_start(out=wb2[:], in_=wvT_r[:, :, n * 512 + 256:(n + 1) * 512])
            for m in range(NJ):
                xb2 = pw.tile([128, KC, 128], F32R, tag="pw")
                nc.sync.dma_start(out=xb2[:], in_=xvT_r[:, :, m * 128:(m + 1) * 128])
                ps = pp.tile([128, 512], F32, tag="pp")
                nc.tensor.matmul(ps[:, 0:256], ones1[0:1, :],
                                 bv_sb[0:1, n * 512:n * 512 + 256],
                                 start=True, stop=False)
                for kc in range(KC):
                    nc.tensor.matmul(ps[:, 0:256], xb2[:, kc, :], wa[:, kc, :],
                                     start=False, stop=(kc == KC - 1))
                nc.tensor.matmul(ps[:, 256:512], ones1[0:1, :],
                                 bv_sb[0:1, n * 512 + 256:(n + 1) * 512],
                                 start=True, stop=False)
                for kc in range(KC):
                    nc.tensor.matmul(ps[:, 256:512], xb2[:, kc, :], wb2[:, kc, :],
                                     start=False, stop=(kc == KC - 1))
                nc.vector.tensor_copy(
                    vpa[m].rearrange("p (h c) -> p h c", c=65)
                    [:, 8 * n:8 * (n + 1), 0:64], ps[:])

    if dbg:
        for m in range(KC):
            nc.sync.dma_start(out=qpT_dbg[m * 128:(m + 1) * 128, :], in_=qpT[m][:])
            nc.sync.dma_start(out=kpT_dbg[m * 128:(m + 1) * 128, :], in_=kpT[m][:])
        for m in range(NJ):
            nc.sync.dma_start(out=vpa_dbg[m * 128:(m + 1) * 128, :], in_=vpa[m][:])

    # ---- phase 2: attention ---------------------------------------------
    with ExitStack() as attn_ctx:
        psA = attn_ctx.enter_context(tc.tile_pool(name="psA", bufs=2, space="PSUM"))
        psO = attn_ctx.enter_context(tc.tile_pool(name="psO", bufs=2, space="PSUM"))
        pe = attn_ctx.enter_context(tc.tile_pool(name="pe", bufs=3))
        pst = attn_ctx.enter_context(tc.tile_pool(name="pst", bufs=1))
        prc = attn_ctx.enter_context(tc.tile_pool(name="prc", bufs=1))

        for p in range(NPAIR):
            hA, hB = 2 * p, 2 * p + 1
            for ic in range(NI):
                isl = slice(ic * 512, (ic + 1) * 512)
                # rows 0-63: head output; row 64: Z (from vpa's ones column)
                ozA = psO.tile([65, 512], F32, tag="o")
                ozB = psO.tile([65, 512], F32, tag="o")
                jbase = 0
                for g, gs in enumerate(JGROUPS):
                    sA = psA.tile([128, 1536], F32, tag="s")
                    sB = psA.tile([128, 1536], F32, tag="s")
                    for gg in range(gs):
                        jc = jbase + gg
                        jsl = slice(jc * 128, (jc + 1) * 128)
                        gsl = slice(gg * 512, (gg + 1) * 512)
                        nc.tensor.matmul(sA[:, gsl], kpT[p][0:64, jsl],
                                         qpT[p][0:64, isl],
                                         tile_position=(0, 0))
                        nc.tensor.matmul(sB[:, gsl], kpT[p][64:128, jsl],
                                         qpT[p][64:128, isl],
                                         tile_position=(64, 0))
                    eA = pe.tile([128, 1536], F32R, tag="e")
                    eB = pe.tile([128, 1536], F32R, tag="e")
                    nc.scalar.activation(eA[:, 0:gs * 512], sA[:, 0:gs * 512],
                                         mybir.ActivationFunctionType.Exp)
                    nc.scalar.activation(eB[:, 0:gs * 512], sB[:, 0:gs * 512],
                                         mybir.ActivationFunctionType.Exp)
                    if dbg and p == 0 and ic == 0 and g == 0:
                        nc.sync.dma_start(out=e_dbg[:], in_=eA[:])
                    first = (g == 0)
                    for gg in range(gs):
                        jc = jbase + gg
                        last = (jc == NJ - 1)
                        gsl = slice(gg * 512, (gg + 1) * 512)
                        nc.tensor.matmul(ozA[:, :],
                                         vpa[jc][:, hA * 65:(hA + 1) * 65],
                                         eA[:, gsl],
                                         start=(first and gg == 0), stop=last)
                        nc.tensor.matmul(ozB[:, :],
                                         vpa[jc][:, hB * 65:(hB + 1) * 65],
                                         eB[:, gsl],
                                         start=(first and gg == 0), stop=last)
                    jbase += gs

                # Z rows live at psum partition 64; move to partitions 0/1 of
                # one SBUF tile via (partition-base-aligned) DVE copy + DMA.
                zt = prc.tile([65, 1024], F32, tag="zt")
                nc.vector.tensor_copy(zt[64:65, 0:512], ozA[64:65, :])
                nc.vector.tensor_copy(zt[64:65, 512:1024], ozB[64:65, :])
                zr = prc.tile([2, 512], F32, tag="zr")
                nc.sync.dma_start(out=zr[0:1, :], in_=zt[64:65, 0:512])
                nc.sync.dma_start(out=zr[1:2, :], in_=zt[64:65, 512:1024])
                recip = prc.tile([2, 512], F32R, tag="rc")
                with nc.allow_low_precision(reason="fp32r rounding of 1/Z"):
                    nc.vector.reciprocal(recip[:], zr[:])
                # broadcast 1/Z across 64 partitions: K=2 selector matmuls
                rA = psA.tile([64, 512], F32, tag="s")
                rB = psA.tile([64, 512], F32, tag="s")
                nc.tensor.matmul(rA[:, :], sel2[0:2, 0:64], recip[:])
                nc.tensor.matmul(rB[:, :], sel2[0:2, 64:128], recip[:])
                rsbA = pst.tile([64, 512], F32, tag="rsb")
                rsbB = pst.tile([64, 512], F32, tag="rsb2")
                nc.vector.tensor_copy(rsbA[:], rA[:, :])
                nc.vector.tensor_copy(rsbB[:], rB[:, :])
                stA = pst.tile([64, 512], F32R, tag="st")
                stB = pst.tile([64, 512], F32R, tag="st2")
                nc.vector.tensor_mul(stA[:], ozA[0:64, :], rsbA[:])
                nc.vector.tensor_mul(stB[:], ozB[0:64, :], rsbB[:])
                nc.sync.dma_start(
                    out=outT_d[p * 128:p * 128 + 64, isl], in_=stA[:])
                nc.sync.dma_start(
                    out=outT_d[p * 128 + 64:p * 128 + 128, isl], in_=stB[:])
                if dbg:
                    nc.sync.dma_start(
                        out=outT_dbg[p * 128:p * 128 + 64, isl], in_=stA[:])
                    nc.sync.dma_start(
                        out=outT_dbg[p * 128 + 64:p * 128 + 128, isl],
                        in_=stB[:])
                    if p == 0 and ic == 0:
                        nc.sync.dma_start(out=z_dbg[:], in_=zr[:])

    # ---- phase 3: output projection -------------------------------------
    with ExitStack() as fin_ctx:
        fw = fin_ctx.enter_context(tc.tile_pool(name="fw", bufs=1))
        fo = fin_ctx.enter_context(tc.tile_pool(name="fo", bufs=3))
        fs = fin_ctx.enter_context(tc.tile_pool(name="fs", bufs=3))
        pf = fin_ctx.enter_context(tc.tile_pool(name="pf", bufs=4, space="PSUM"))

        for n in range(D // 512):
            nsl = slice(n * 512, (n + 1) * 512)
            wob = fw.tile([128, KC, 512], F32R, tag="fw")
            nc.sync.dma_start(out=wob[:], in_=woT_r[:, :, nsl])
            for m in range(MQ):
                otb = fo.tile([128, KC, 128], F32R, tag="fo")
                nc.sync.dma_start(out=otb[:],
                                  in_=outT_r[:, :, m * 128:(m + 1) * 128])
                ps = pf.tile([128, 512], F32, tag="pf")
                nc.tensor.matmul(ps[:], ones1[0:1, :], bo_sb[0:1, nsl],
                                 start=True, stop=False)
                for kc in range(KC):
                    nc.tensor.matmul(ps[:], otb[:, kc, :], wob[:, kc, :],
                                     start=False, stop=(kc == KC - 1))
                ost = fs.tile([128, 512], F32, tag="fs")
                nc.scalar.copy(ost[:], ps[:])
                nc.sync.dma_start(out=out[m * 128:(m + 1) * 128, nsl],
                                  in_=ost[:])


_NC_CACHE = None


def _get_program():
    global _NC_CACHE
    if _NC_CACHE is None:
        _NC_CACHE = build_program()
    return _NC_CACHE


def prep_in_maps(q, k, v, w_q, b_q, w_k, b_k, w_v, b_v, w_o, b_o):
    f = np.float32
    q, k, v = (np.asarray(t, f) for t in (q, k, v))
    scale = 1.0 / np.sqrt(DH)
    wqT = np.ascontiguousarray((np.asarray(w_q, f) * scale).T)
    wkT = np.ascontiguousarray(np.asarray(w_k, f).T)
    wvT = np.ascontiguousarray(np.asarray(w_v, f).T)
    woT = np.ascontiguousarray(np.asarray(w_o, f).T)
    bqc = np.ascontiguousarray((np.asarray(b_q, f) * scale).reshape(KC, 128).T)
    bkc = np.ascontiguousarray(np.asarray(b_k, f).reshape(KC, 128).T)
    bvr = np.asarray(b_v, f).reshape(1, D)
    bor = np.asarray(b_o, f).reshape(1, D)
    c_or = np.ones((1, 128), f)
    c_oc = np.ones((128, 1), f)
    c_sel = np.zeros((2, 128), f)
    c_sel[0, 0:64] = 1.0
    c_sel[1, 64:128] = 1.0
    c_vone = np.ones((128, NH), f)
    in_maps = []
    for c in range(N_CORES):
        b, qh = c // 2, c % 2
        kTb = np.ascontiguousarray(k[b].T)
        vTb = np.ascontiguousarray(v[b].T)
        qTb = np.ascontiguousarray(q[b].T[:, qh * LQ:(qh + 1) * LQ])
        in_maps.append({
            "xqT": qTb, "xkT": kTb, "xvT": vTb,
            "wqT": wqT, "wkT": wkT, "wvT": wvT, "woT": woT,
            "bqc": bqc, "bkc": bkc, "bvr": bvr, "bor": bor,
            "c_or": c_or, "c_oc": c_oc, "c_sel": c_sel, "c_vone": c_vone,
        })
    return in_maps


def run(in_maps, trace=False, **kw):
    nc = _get_program()
    return run_bass_kernel_spmd(nc, in_maps, list(range(N_CORES)),
                                trace=trace, **kw)


def kernel(**inputs):
    in_maps = prep_in_maps(**inputs)
    res = run(in_maps)
    out = np.empty((B, L, D), np.float32)
    for c in range(N_CORES):
        b, qh = c // 2, c % 2
        out[b, qh * LQ:(qh + 1) * LQ, :] = res.results[c]["out"]
    return out



# revision 19
# speedup vs baseline: 1.6907x; 1.6907x over previous
"""Multi-head attention (B=4, L=2048, D=1024, H=16) on 8 Trainium2 NeuronCores.

Sharding: core c = (batch b = c//2, query-half qh = c%2). Each core computes
all 16 heads for its 1024 query rows against the full 2048 keys/values of its
batch. Fully SPMD, no collectives.

v3 design:
  - bf16 matmul operands (fp32 PSUM accumulation); e-tiles f32r (bf16 ACT
    output measured 1.5 cyc/elem vs 1.0 for fp32).
  - Weights SBUF-resident, loaded once; weight-stationary projections with
    paired moving chunks (one LDWEIGHTS per two matmuls).
  - p-major attention: kpT[p] produced just-in-time (pool bufs=2), used by
    units (p,0),(p,1); projection/out-projection work interleaved between
    units to keep the PE dense while ACT (exp) runs.
  - Z via ones-column in vpa -> PSUM row 64; Z broadcast to 64 partitions by
    a K=1 matmul, then reciprocal_approx_fast on the base-0 SBUF copy
    (the custom DVE op mis-reads partition-base-64 PSUM APs).
  - outT kept in SBUF; head-B rows shifted to partitions 64:128 via
    SBUF->SBUF DMA on the gpsimd queue.
  - x-chunk DMAs issued from gpsimd, weights from sync: two queues, no
    head-of-line blocking of the first x chunks behind 8MB of weights.
"""

import sys

if "/opt/trn_rl_repo" not in sys.path:
    sys.path.insert(0, "/opt/trn_rl_repo")

import numpy as np

import concourse.bacc as bacc
import concourse.tile as tile
from concourse import mybir
from concourse.bass_utils import run_bass_kernel_spmd

N_CORES = 8
B, L, D = 4, 2048, 1024
NH, DH = 16, 64          # heads, head dim
LQ = L // 2              # query rows per core
F32 = mybir.dt.float32
F32R = mybir.dt.float32r
BF16 = mybir.dt.bfloat16

KC = D // 128            # 8 contraction chunks for projections
NJ = L // 128            # 16 key j-chunks
NPAIR = NH // 2          # 8 head pairs


def build_program(dbg=False):
    nc = bacc.Bacc("TRN2", target_bir_lowering=False, debug=False,
                   num_devices=N_CORES)
    with tile.TileContext(nc) as tc:
        _emit(nc, tc, dbg=dbg)
    nc.compile()
    return nc


def _emit(nc, tc, dbg=False):
    from contextlib import ExitStack

    top = ExitStack()
    dram = top.enter_context(tc.tile_pool(name="dram", bufs=1, space="DRAM"))
    xqT = dram.tile([D, LQ], BF16, kind="ExternalInput", name="xqT", uniquify=False)
    xkT = dram.tile([D, L], BF16, kind="ExternalInput", name="xkT", uniquify=False)
    xvT = dram.tile([D, L], BF16, kind="ExternalInput", name="xvT", uniquify=False)
    wqT = dram.tile([D, D], BF16, kind="ExternalInput", name="wqT", uniquify=False)
    wkT = dram.tile([D, D], BF16, kind="ExternalInput", name="wkT", uniquify=False)
    wvT = dram.tile([D, D], BF16, kind="ExternalInput", name="wvT", uniquify=False)
    woT = dram.tile([D, D], BF16, kind="ExternalInput", name="woT", uniquify=False)
    bqc = dram.tile([128, KC], F32, kind="ExternalInput", name="bqc", uniquify=False)
    bkc = dram.tile([128, KC], F32, kind="ExternalInput", name="bkc", uniquify=False)
    bvr = dram.tile([1, D], BF16, kind="ExternalInput", name="bvr", uniquify=False)
    bor = dram.tile([1, D], BF16, kind="ExternalInput", name="bor", uniquify=False)
    out = dram.tile([LQ, D], F32, kind="ExternalOutput", name="out", uniquify=False)
    if dbg:
        d_kp = dram.tile([128, L], BF16, kind="ExternalOutput", name="d_kp",
                         uniquify=False)
        d_qp = dram.tile([128, LQ], BF16, kind="ExternalOutput", name="d_qp",
                         uniquify=False)
        d_vpa = dram.tile([128, NH * 65], F32R, kind="ExternalOutput",
                          name="d_vpa", uniquify=False)
        d_e = dram.tile([128, 1024], F32, kind="ExternalOutput", name="d_e",
                        uniquify=False)
        d_oz = dram.tile([130, 512], F32, kind="ExternalOutput", name="d_oz",
                         uniquify=False)
        d_z = dram.tile([2, 512], F32, kind="ExternalOutput", name="d_z",
                        uniquify=False)
        d_outT = dram.tile([128, LQ], BF16, kind="ExternalOutput", name="d_outT",
                           uniquify=False)

    xkT_r = xkT.rearrange("(kc p) l -> p kc l", p=128)
    xqT_r = xqT.rearrange("(kc p) l -> p kc l", p=128)
    xvT_r = xvT.rearrange("(kc p) l -> p kc l", p=128)
    wqT_r = wqT.rearrange("(kc p) m -> p kc m", p=128)
    wkT_r = wkT.rearrange("(kc p) m -> p kc m", p=128)
    wvT_r = wvT.rearrange("(kc p) m -> p kc m", p=128)
    woT_r = woT.rearrange("(kc p) m -> p kc m", p=128)

    # ---- persistent SBUF ------------------------------------------------
    pers = top.enter_context(tc.tile_pool(name="pers", bufs=1))
    wk = pers.tile([128, KC, D], BF16, name="wk")
    wq = pers.tile([128, KC, D], BF16, name="wq")
    qpT = [pers.tile([128, LQ], BF16, name=f"qpT{m}") for m in range(KC)]
    # vpa: per j-chunk, 16 heads x (64 value cols + 1 ones col -> Z via PV)
    vpa = [pers.tile([128, NH * 65], F32R, name=f"vpa{m}") for m in range(NJ)]
    outT = [pers.tile([128, LQ], BF16, name=f"outT{m}") for m in range(KC)]
    bq_sb = pers.tile([128, KC], F32, name="bq_sb")
    bk_sb = pers.tile([128, KC], F32, name="bk_sb")
    bv_sb = pers.tile([1, D], BF16, name="bv_sb")
    bo_sb = pers.tile([1, D], BF16, name="bo_sb")
    cones = pers.tile([128, 128], BF16, name="cones")
    cones_fr = pers.tile([128, 64], F32R, name="cones_fr")

    # ---- working pools --------------------------------------------------
    pwvo = top.enter_context(tc.tile_pool(name="pwvo", bufs=1))
    wv = pwvo.tile([128, KC, D], BF16, tag="wvo")
    wo = []
    pkq = top.enter_context(tc.tile_pool(name="pkq", bufs=2))
    px = top.enter_context(tc.tile_pool(name="px", bufs=3))
    pe_ = top.enter_context(tc.tile_pool(name="pe", bufs=2))
    pz = top.enter_context(tc.tile_pool(name="pz", bufs=1))
    pf = top.enter_context(tc.tile_pool(name="pf", bufs=2))
    psS = top.enter_context(tc.tile_pool(name="psS", bufs=2, space="PSUM"))
    psO = top.enter_context(tc.tile_pool(name="psO", bufs=1, space="PSUM"))
    psX = top.enter_context(tc.tile_pool(name="psX", bufs=2, space="PSUM"))

    # ---- constants / weight loads (weights on sync queue, x on gpsimd) --
    nc.vector.memset(cones[:], 1.0)
    nc.vector.tensor_copy(cones_fr[:], cones[:, 0:64])
    for m in range(NJ):
        nc.vector.tensor_copy(
            vpa[m].rearrange("p (h c) -> p h c", c=65)[:, :, 64:65],
            cones[:, 0:NH])
    nc.sync.dma_start(out=wv[:], in_=wvT_r[:])
    nc.sync.dma_start(out=bv_sb[:], in_=bvr[:])
    nc.sync.dma_start(out=wk[:], in_=wkT_r[:])
    nc.sync.dma_start(out=bk_sb[:], in_=bkc[:])
    nc.sync.dma_start(out=wq[:], in_=wqT_r[:])
    nc.sync.dma_start(out=bq_sb[:], in_=bqc[:])
    nc.sync.dma_start(out=bo_sb[:], in_=bor[:])

    # ---- emit helpers ---------------------------------------------------
    def vp_block(n, c):
        """Value projection: heads 8n..8n+7 cols for l-chunk c (j = 4c..4c+3)."""
        xv = px.tile([128, KC, 512], BF16, tag="px")
        nc.gpsimd.dma_start(out=xv[:], in_=xvT_r[:, :, c * 512:(c + 1) * 512])
        nsl = slice(n * 512, (n + 1) * 512)
        for mm in range(4):
            m = 4 * c + mm
            ps = psX.tile([128, 512], F32, tag="aux")
            nc.tensor.matmul(ps[:], cones[0:1, :], bv_sb[0:1, nsl],
                             start=True, stop=False)
            for kc in range(KC):
                nc.tensor.matmul(ps[:], xv[:, kc, mm * 128:(mm + 1) * 128],
                                 wv[:, kc, nsl], start=False, stop=(kc == KC - 1))
            nc.vector.tensor_copy(
                vpa[m].rearrange("p (h c) -> p h c", c=65)
                [:, 8 * n:8 * (n + 1), 0:64], ps[:])

    def kpT_block(p):
        """Key projection rows for head pair p: kpT[p][dh128, L].

        kc-outer with two moving chunks per stationary weight block: one
        LDWEIGHTS serves two matmuls, and consecutive matmuls alternate
        PSUM banks."""
        kp = pkq.tile([128, L], BF16, tag="kpT")
        psl = slice(p * 128, (p + 1) * 128)
        for h in range(2):
            c0, c1 = 2 * h, 2 * h + 1
            xk0 = px.tile([128, KC, 512], BF16, tag="px")
            nc.gpsimd.dma_start(out=xk0[:], in_=xkT_r[:, :, c0 * 512:(c0 + 1) * 512])
            xk1 = px.tile([128, KC, 512], BF16, tag="px")
            nc.gpsimd.dma_start(out=xk1[:], in_=xkT_r[:, :, c1 * 512:(c1 + 1) * 512])
            ps0 = psX.tile([128, 512], F32, tag="aux")
            ps1 = psX.tile([128, 512], F32, tag="aux")
            for kc in range(KC):
                nc.tensor.matmul(ps0[:], wk[:, kc, psl], xk0[:, kc, :],
                                 start=(kc == 0), stop=(kc == KC - 1))
                nc.tensor.matmul(ps1[:], wk[:, kc, psl], xk1[:, kc, :],
                                 start=(kc == 0), stop=(kc == KC - 1))
            nc.vector.tensor_scalar_add(
                kp[:, c0 * 512:(c0 + 1) * 512], ps0[:], bk_sb[:, p:p + 1])
            nc.vector.tensor_scalar_add(
                kp[:, c1 * 512:(c1 + 1) * 512], ps1[:], bk_sb[:, p:p + 1])
        return kp

    def qpT_all():
        xq0 = px.tile([128, KC, 512], BF16, tag="px")
        nc.gpsimd.dma_start(out=xq0[:], in_=xqT_r[:, :, 0:512])
        xq1 = px.tile([128, KC, 512], BF16, tag="px")
        nc.gpsimd.dma_start(out=xq1[:], in_=xqT_r[:, :, 512:1024])
        for p in range(KC):
            psl = slice(p * 128, (p + 1) * 128)
            ps0 = psX.tile([128, 512], F32, tag="aux")
            ps1 = psX.tile([128, 512], F32, tag="aux")
            for kc in range(KC):
                nc.tensor.matmul(ps0[:], wq[:, kc, psl], xq0[:, kc, :],
                                 start=(kc == 0), stop=(kc == KC - 1))
                nc.tensor.matmul(ps1[:], wq[:, kc, psl], xq1[:, kc, :],
                                 start=(kc == 0), stop=(kc == KC - 1))
            nc.vector.tensor_scalar_add(qpT[p][:, 0:512], ps0[:], bq_sb[:, p:p + 1])
            nc.vector.tensor_scalar_add(qpT[p][:, 512:1024], ps1[:],
                                        bq_sb[:, p:p + 1])

    def outproj_block(i):
        """out[i*128:(i+1)*128, :] = outT^T @ woT + bo (both 512-col halves)."""
        il = slice(i * 128, (i + 1) * 128)
        ps0 = psX.tile([128, 512], F32, tag="aux")
        ps1 = psX.tile([128, 512], F32, tag="aux")
        nc.tensor.matmul(ps0[:], cones[0:1, :], bo_sb[0:1, 0:512],
                         start=True, stop=False)
        nc.tensor.matmul(ps1[:], cones[0:1, :], bo_sb[0:1, 512:1024],
                         start=True, stop=False)
        for kc in range(KC):
            nc.tensor.matmul(ps0[:], outT[kc][:, il], wo[0][:, kc, 0:512],
                             start=False, stop=(kc == KC - 1))
            nc.tensor.matmul(ps1[:], outT[kc][:, il], wo[0][:, kc, 512:1024],
                             start=False, stop=(kc == KC - 1))
        f0 = pf.tile([128, 512], F32, tag="fout")
        nc.vector.tensor_copy(f0[:], ps0[:])
        nc.gpsimd.dma_start(out=out[il, 0:512], in_=f0[:])
        f1 = pf.tile([128, 512], F32, tag="fout")
        nc.vector.tensor_copy(f1[:], ps1[:])
        nc.gpsimd.dma_start(out=out[il, 512:1024], in_=f1[:])

    def unit(p, ic, kp):
        """Attention for head pair p, query chunk ic (512 rows)."""
        hA, hB = 2 * p, 2 * p + 1
        isl = slice(ic * 512, (ic + 1) * 512)
        # rows 0-63: head output; row 64: Z (from vpa's ones column)
        ozA = psO.tile([65, 512], F32, tag="ozA")
        ozB = psO.tile([65, 512], F32, tag="ozB")
        for j in range(NJ):
            jsl = slice(j * 128, (j + 1) * 128)
            s = psS.tile([128, 1024], F32, tag="s")
            nc.tensor.matmul(s[:, 0:512], kp[0:64, jsl], qpT[p][0:64, isl],
                             tile_position=(0, 0))
            nc.tensor.matmul(s[:, 512:1024], kp[64:128, jsl],
                             qpT[p][64:128, isl], tile_position=(64, 0))
            e = pe_.tile([128, 1024], F32R, tag="e")
            nc.scalar.activation(e[:], s[:], mybir.ActivationFunctionType.Exp)
            if dbg and p == 0 and ic == 0 and j == 0:
                nc.gpsimd.dma_start(out=d_e[:], in_=e[:])
            nc.tensor.matmul(ozA[:], vpa[j][:, hA * 65:(hA + 1) * 65],
                             e[:, 0:512], start=(j == 0), stop=(j == NJ - 1))
            nc.tensor.matmul(ozB[:], vpa[j][:, hB * 65:(hB + 1) * 65],
                             e[:, 512:1024], start=(j == 0), stop=(j == NJ - 1))
        if dbg and p == 0 and ic == 0:
            ozc = pz.tile([65, 1024], F32, tag="zb")
            nc.vector.tensor_copy(ozc[:, 0:512], ozA[:])
            nc.vector.tensor_copy(ozc[:, 512:1024], ozB[:])
            nc.sync.dma_start(out=d_oz[0:65, :], in_=ozc[:, 0:512])
            nc.sync.dma_start(out=d_oz[65:130, :], in_=ozc[:, 512:1024])
        # Z rows live at psum partition 64. Broadcast Z to partitions 0:64
        # via a K=1 matmul (reads partition 64), then take 1/Z on the
        # resulting base-0 SBUF tile.
        zb = pz.tile([65, 1024], F32R, tag="zb")
        nc.vector.tensor_copy(zb[64:65, 0:512], ozA[64:65, :])
        nc.vector.tensor_copy(zb[64:65, 512:1024], ozB[64:65, :])
        rA = psX.tile([64, 512], F32, tag="aux")
        nc.tensor.matmul(rA[:], cones_fr[64:65, :], zb[64:65, 0:512])
        rsbA = pz.tile([64, 512], F32, tag="rsbA")
        nc.vector.tensor_copy(rsbA[:], rA[:])
        riA = pz.tile([64, 512], F32, tag="riA")
        nc.vector.reciprocal_approx_fast(riA[:], rsbA[:])
        if dbg and p == 0 and ic == 0:
            nc.sync.dma_start(out=d_z[0:1, :], in_=riA[0:1, :])
        nc.vector.tensor_mul(outT[p][0:64, isl], ozA[0:64, :], riA[:])
        rB = psX.tile([64, 512], F32, tag="aux")
        nc.tensor.matmul(rB[:], cones_fr[64:65, :], zb[64:65, 512:1024])
        rsbB = pz.tile([64, 512], F32, tag="rsbB")
        nc.vector.tensor_copy(rsbB[:], rB[:])
        riB = pz.tile([64, 512], F32, tag="riB")
        nc.vector.reciprocal_approx_fast(riB[:], rsbB[:])
        stB = pz.tile([64, 512], BF16, tag="stB")
        nc.vector.tensor_mul(stB[:], ozB[0:64, :], riB[:])
        # head B's rows live at partitions 0:64; shift to outT rows 64:128
        nc.gpsimd.dma_start(out=outT[p][64:128, isl], in_=stB[:])

    # ---- emission schedule (p-major, fillers keep the PE dense) ---------
    for c in range(4):
        vp_block(0, c)
    qpT_all()
    kp_tiles = {0: kpT_block(0)}

    def load_wo():
        wo_t = pwvo.tile([128, KC, D], BF16, tag="wvo")
        nc.sync.dma_start(out=wo_t[:], in_=woT_r[:])
        wo.append(wo_t)

    fillers = {
        1: [lambda: kp_tiles.__setitem__(1, kpT_block(1))],
        2: [lambda: kp_tiles.__setitem__(2, kpT_block(2)),
            lambda: vp_block(1, 0), lambda: vp_block(1, 1)],
        3: [lambda: kp_tiles.__setitem__(3, kpT_block(3)),
            lambda: vp_block(1, 2), lambda: vp_block(1, 3)],
        4: [lambda: kp_tiles.__setitem__(4, kpT_block(4)), load_wo],
        5: [lambda: kp_tiles.__setitem__(5, kpT_block(5))],
        6: [lambda: kp_tiles.__setitem__(6, kpT_block(6))],
        7: [lambda: kp_tiles.__setitem__(7, kpT_block(7))],
    }

    for p in range(NPAIR):
        for fn in fillers.get(p, []):
            fn()
        if dbg and p == 0:
            nc.sync.dma_start(out=d_kp[:], in_=kp_tiles[0][:])
            nc.sync.dma_start(out=d_qp[:], in_=qpT[0][:])
            nc.sync.dma_start(out=d_vpa[:], in_=vpa[0][:])
        unit(p, 0, kp_tiles[p])
        if p == NPAIR - 1:
            # outT cols 0:512 (ic=0) complete: fill unit(7,1)'s ACT-bound
            # window with the first half of the out-projection (out row
            # blocks i<4 contract only outT[:, 0:512])
            for i in range(4):
                outproj_block(i)
        unit(p, 1, kp_tiles[p])
    if dbg:
        nc.sync.dma_start(out=d_outT[:], in_=outT[0][:])

    for i in range(4, KC):
        outproj_block(i)

    top.close()


_NC_CACHE = None


def _get_program():
    global _NC_CACHE
    if _NC_CACHE is None:
        _NC_CACHE = build_program()
    return _NC_CACHE


def prep_in_maps(q, k, v, w_q, b_q, w_k, b_k, w_v, b_v, w_o, b_o):
    import ml_dtypes

    f = np.float32
    bf = ml_dtypes.bfloat16
    q, k, v = (np.asarray(t, f) for t in (q, k, v))
    scale = 1.0 / np.sqrt(DH)
    wqT = np.ascontiguousarray((np.asarray(w_q, f) * scale).T.astype(bf))
    wkT = np.ascontiguousarray(np.asarray(w_k, f).T.astype(bf))
    wvT = np.ascontiguousarray(np.asarray(w_v, f).T.astype(bf))
    woT = np.ascontiguousarray(np.asarray(w_o, f).T.astype(bf))
    bqc = np.ascontiguousarray((np.asarray(b_q, f) * scale).reshape(KC, 128).T)
    bkc = np.ascontiguousarray(np.asarray(b_k, f).reshape(KC, 128).T)
    bvr = np.asarray(b_v, f).reshape(1, D).astype(bf)
    bor = np.asarray(b_o, f).reshape(1, D).astype(bf)
    in_maps = []
    for c in range(N_CORES):
        b, qh = c // 2, c % 2
        kTb = np.ascontiguousarray(k[b].T.astype(bf))
        vTb = np.ascontiguousarray(v[b].T.astype(bf))
        qTb = np.ascontiguousarray(q[b].T[:, qh * LQ:(qh + 1) * LQ].astype(bf))
        in_maps.append({
            "xqT": qTb, "xkT": kTb, "xvT": vTb,
            "wqT": wqT, "wkT": wkT, "wvT": wvT, "woT": woT,
            "bqc": bqc, "bkc": bkc, "bvr": bvr, "bor": bor,
        })
    return in_maps


def run(in_maps, trace=False, **kw):
    nc = _get_program()
    return run_bass_kernel_spmd(nc, in_maps, list(range(N_CORES)),
                                trace=trace, **kw)


def kernel(**inputs):
    in_maps = prep_in_maps(**inputs)
    res = run(in_maps)
    out = np.empty((B, L, D), np.float32)
    for c in range(N_CORES):
        b, qh = c // 2, c % 2
        out[b, qh * LQ:(qh + 1) * LQ, :] = res.results[c]["out"]
    return out


# revision 20
# speedup vs baseline: 1.7385x; 1.0282x over previous
"""Multi-head attention (B=4, L=2048, D=1024, H=16) on 8 Trainium2 NeuronCores.

Sharding: core c = (batch b = c//2, query-half qh = c%2). Each core computes
all 16 heads for its 1024 query rows against the full 2048 keys/values of its
batch. Fully SPMD, no collectives.

v3 design:
  - bf16 matmul operands (fp32 PSUM accumulation); e-tiles f32r (bf16 ACT
    output measured 1.5 cyc/elem vs 1.0 for fp32).
  - Weights SBUF-resident, loaded once; weight-stationary projections with
    paired moving chunks (one LDWEIGHTS per two matmuls).
  - p-major attention: kpT[p] produced just-in-time (pool bufs=2), used by
    units (p,0),(p,1); projection/out-projection work interleaved between
    units to keep the PE dense while ACT (exp) runs.
  - Z via ones-column in vpa -> PSUM row 64; Z broadcast to 64 partitions by
    a K=1 matmul, then reciprocal_approx_fast on the base-0 SBUF copy
    (the custom DVE op mis-reads partition-base-64 PSUM APs).
  - outT kept in SBUF; head-B rows shifted to partitions 64:128 via
    SBUF->SBUF DMA on the gpsimd queue.
  - x-chunk DMAs issued from gpsimd, weights from sync: two queues, no
    head-of-line blocking of the first x chunks behind 8MB of weights.
"""

import sys

if "/opt/trn_rl_repo" not in sys.path:
    sys.path.insert(0, "/opt/trn_rl_repo")

import numpy as np

import concourse.bacc as bacc
import concourse.tile as tile
from concourse import mybir
from concourse.bass_utils import run_bass_kernel_spmd

N_CORES = 8
B, L, D = 4, 2048, 1024
NH, DH = 16, 64          # heads, head dim
LQ = L // 2              # query rows per core
F32 = mybir.dt.float32
F32R = mybir.dt.float32r
BF16 = mybir.dt.bfloat16

KC = D // 128            # 8 contraction chunks for projections
NJ = L // 128            # 16 key j-chunks
NPAIR = NH // 2          # 8 head pairs


def build_program(dbg=False):
    nc = bacc.Bacc("TRN2", target_bir_lowering=False, debug=False,
                   num_devices=N_CORES)
    with tile.TileContext(nc) as tc:
        _emit(nc, tc, dbg=dbg)
    nc.compile()
    return nc


def _emit(nc, tc, dbg=False):
    from contextlib import ExitStack

    top = ExitStack()
    dram = top.enter_context(tc.tile_pool(name="dram", bufs=1, space="DRAM"))
    xqT = dram.tile([D, LQ], BF16, kind="ExternalInput", name="xqT", uniquify=False)
    xkT = dram.tile([D, L], BF16, kind="ExternalInput", name="xkT", uniquify=False)
    xvT = dram.tile([D, L], BF16, kind="ExternalInput", name="xvT", uniquify=False)
    wqT = dram.tile([D, D], BF16, kind="ExternalInput", name="wqT", uniquify=False)
    wkT = dram.tile([D, D], BF16, kind="ExternalInput", name="wkT", uniquify=False)
    wvT = dram.tile([D, D], BF16, kind="ExternalInput", name="wvT", uniquify=False)
    woT = dram.tile([D, D], BF16, kind="ExternalInput", name="woT", uniquify=False)
    bqc = dram.tile([128, KC], F32, kind="ExternalInput", name="bqc", uniquify=False)
    bkc = dram.tile([128, KC], F32, kind="ExternalInput", name="bkc", uniquify=False)
    bvr = dram.tile([1, D], BF16, kind="ExternalInput", name="bvr", uniquify=False)
    bor = dram.tile([1, D], BF16, kind="ExternalInput", name="bor", uniquify=False)
    out = dram.tile([LQ, D], F32, kind="ExternalOutput", name="out", uniquify=False)
    if dbg:
        d_kp = dram.tile([128, L], BF16, kind="ExternalOutput", name="d_kp",
                         uniquify=False)
        d_qp = dram.tile([128, LQ], BF16, kind="ExternalOutput", name="d_qp",
                         uniquify=False)
        d_vpa = dram.tile([128, NH * 65], BF16, kind="ExternalOutput",
                          name="d_vpa", uniquify=False)
        d_e = dram.tile([128, 1024], F32, kind="ExternalOutput", name="d_e",
                        uniquify=False)
        d_oz = dram.tile([130, 512], F32, kind="ExternalOutput", name="d_oz",
                         uniquify=False)
        d_z = dram.tile([2, 512], F32, kind="ExternalOutput", name="d_z",
                        uniquify=False)
        d_outT = dram.tile([128, LQ], BF16, kind="ExternalOutput", name="d_outT",
                           uniquify=False)

    xkT_r = xkT.rearrange("(kc p) l -> p kc l", p=128)
    xqT_r = xqT.rearrange("(kc p) l -> p kc l", p=128)
    xvT_r = xvT.rearrange("(kc p) l -> p kc l", p=128)
    wqT_r = wqT.rearrange("(kc p) m -> p kc m", p=128)
    wkT_r = wkT.rearrange("(kc p) m -> p kc m", p=128)
    wvT_r = wvT.rearrange("(kc p) m -> p kc m", p=128)
    woT_r = woT.rearrange("(kc p) m -> p kc m", p=128)

    # ---- persistent SBUF ------------------------------------------------
    pers = top.enter_context(tc.tile_pool(name="pers", bufs=1))
    wk = pers.tile([128, KC, D], BF16, name="wk")
    wq = pers.tile([128, KC, D], BF16, name="wq")
    qpT = [pers.tile([128, LQ], BF16, name=f"qpT{m}") for m in range(KC)]
    # vpa: per j-chunk, 16 heads x (64 value cols + 1 ones col -> Z via PV)
    vpa = [pers.tile([128, NH * 65], BF16, name=f"vpa{m}") for m in range(NJ)]
    outT = [pers.tile([128, LQ], BF16, name=f"outT{m}") for m in range(KC)]
    bq_sb = pers.tile([128, KC], F32, name="bq_sb")
    bk_sb = pers.tile([128, KC], F32, name="bk_sb")
    bv_sb = pers.tile([1, D], BF16, name="bv_sb")
    bo_sb = pers.tile([1, D], BF16, name="bo_sb")
    cones = pers.tile([128, 128], BF16, name="cones")
    bv_bc = pers.tile([128, D], BF16, name="bv_bc")
    bo_bc = pers.tile([128, D], BF16, name="bo_bc")

    # ---- working pools --------------------------------------------------
    pwvo = top.enter_context(tc.tile_pool(name="pwvo", bufs=1))
    wv = pwvo.tile([128, KC, D], BF16, tag="wvo")
    wo = []
    pkq = top.enter_context(tc.tile_pool(name="pkq", bufs=2))
    px = top.enter_context(tc.tile_pool(name="px", bufs=3))
    pe_ = top.enter_context(tc.tile_pool(name="pe", bufs=2))
    pz = top.enter_context(tc.tile_pool(name="pz", bufs=1))
    pf = top.enter_context(tc.tile_pool(name="pf", bufs=2))
    psS = top.enter_context(tc.tile_pool(name="psS", bufs=2, space="PSUM"))
    psO = top.enter_context(tc.tile_pool(name="psO", bufs=1, space="PSUM"))
    psX = top.enter_context(tc.tile_pool(name="psX", bufs=2, space="PSUM"))

    # ---- constants / weight loads (weights on sync queue, x on gpsimd) --
    nc.vector.memset(cones[:], 1.0)
    for m in range(NJ):
        nc.vector.tensor_copy(
            vpa[m].rearrange("p (h c) -> p h c", c=65)[:, :, 64:65],
            cones[:, 0:NH])
    nc.sync.dma_start(out=wv[:], in_=wvT_r[:])
    nc.sync.dma_start(out=bv_sb[:], in_=bvr[:])
    nc.sync.dma_start(out=wk[:], in_=wkT_r[:])
    nc.sync.dma_start(out=bk_sb[:], in_=bkc[:])
    nc.sync.dma_start(out=wq[:], in_=wqT_r[:])
    nc.sync.dma_start(out=bq_sb[:], in_=bqc[:])
    nc.sync.dma_start(out=bo_sb[:], in_=bor[:])
    for half in range(2):
        hs = slice(half * 512, (half + 1) * 512)
        psb = psX.tile([128, 512], F32, tag="aux")
        nc.tensor.matmul(psb[:], cones[0:1, :], bv_sb[0:1, hs])
        nc.vector.tensor_copy(bv_bc[:, hs], psb[:])
        psb2 = psX.tile([128, 512], F32, tag="aux")
        nc.tensor.matmul(psb2[:], cones[0:1, :], bo_sb[0:1, hs])
        nc.vector.tensor_copy(bo_bc[:, hs], psb2[:])

    # ---- emit helpers ---------------------------------------------------
    def vp_block(n, c):
        """Value projection: heads 8n..8n+7 cols for l-chunk c (j = 4c..4c+3)."""
        xv = px.tile([128, KC, 512], BF16, tag="px")
        nc.gpsimd.dma_start(out=xv[:], in_=xvT_r[:, :, c * 512:(c + 1) * 512])
        nsl = slice(n * 512, (n + 1) * 512)
        for mm in range(4):
            m = 4 * c + mm
            ps = psX.tile([128, 512], F32, tag="aux")
            for kc in range(KC):
                nc.tensor.matmul(ps[:], xv[:, kc, mm * 128:(mm + 1) * 128],
                                 wv[:, kc, nsl], start=(kc == 0),
                                 stop=(kc == KC - 1))
            nc.vector.tensor_add(
                vpa[m].rearrange("p (h c) -> p h c", c=65)
                [:, 8 * n:8 * (n + 1), 0:64], ps[:], bv_bc[:, nsl])

    def kpT_block(p):
        """Key projection rows for head pair p: kpT[p][dh128, L].

        kc-outer with two moving chunks per stationary weight block: one
        LDWEIGHTS serves two matmuls, and consecutive matmuls alternate
        PSUM banks."""
        kp = pkq.tile([128, L], BF16, tag="kpT")
        psl = slice(p * 128, (p + 1) * 128)
        for h in range(2):
            c0, c1 = 2 * h, 2 * h + 1
            xk0 = px.tile([128, KC, 512], BF16, tag="px")
            nc.gpsimd.dma_start(out=xk0[:], in_=xkT_r[:, :, c0 * 512:(c0 + 1) * 512])
            xk1 = px.tile([128, KC, 512], BF16, tag="px")
            nc.gpsimd.dma_start(out=xk1[:], in_=xkT_r[:, :, c1 * 512:(c1 + 1) * 512])
            ps0 = psX.tile([128, 512], F32, tag="aux")
            ps1 = psX.tile([128, 512], F32, tag="aux")
            for kc in range(KC):
                nc.tensor.matmul(ps0[:], wk[:, kc, psl], xk0[:, kc, :],
                                 start=(kc == 0), stop=(kc == KC - 1))
                nc.tensor.matmul(ps1[:], wk[:, kc, psl], xk1[:, kc, :],
                                 start=(kc == 0), stop=(kc == KC - 1))
            nc.vector.tensor_scalar_add(
                kp[:, c0 * 512:(c0 + 1) * 512], ps0[:], bk_sb[:, p:p + 1])
            nc.vector.tensor_scalar_add(
                kp[:, c1 * 512:(c1 + 1) * 512], ps1[:], bk_sb[:, p:p + 1])
        return kp

    def qpT_all():
        xq0 = px.tile([128, KC, 512], BF16, tag="px")
        nc.gpsimd.dma_start(out=xq0[:], in_=xqT_r[:, :, 0:512])
        xq1 = px.tile([128, KC, 512], BF16, tag="px")
        nc.gpsimd.dma_start(out=xq1[:], in_=xqT_r[:, :, 512:1024])
        for p in range(KC):
            psl = slice(p * 128, (p + 1) * 128)
            ps0 = psX.tile([128, 512], F32, tag="aux")
            ps1 = psX.tile([128, 512], F32, tag="aux")
            for kc in range(KC):
                nc.tensor.matmul(ps0[:], wq[:, kc, psl], xq0[:, kc, :],
                                 start=(kc == 0), stop=(kc == KC - 1))
                nc.tensor.matmul(ps1[:], wq[:, kc, psl], xq1[:, kc, :],
                                 start=(kc == 0), stop=(kc == KC - 1))
            nc.vector.tensor_scalar_add(qpT[p][:, 0:512], ps0[:], bq_sb[:, p:p + 1])
            nc.vector.tensor_scalar_add(qpT[p][:, 512:1024], ps1[:],
                                        bq_sb[:, p:p + 1])

    def outproj_block(i):
        """out[i*128:(i+1)*128, :] = outT^T @ woT + bo (both 512-col halves)."""
        il = slice(i * 128, (i + 1) * 128)
        ps0 = psX.tile([128, 512], F32, tag="aux")
        ps1 = psX.tile([128, 512], F32, tag="aux")
        for kc in range(KC):
            nc.tensor.matmul(ps0[:], outT[kc][:, il], wo[0][:, kc, 0:512],
                             start=(kc == 0), stop=(kc == KC - 1))
            nc.tensor.matmul(ps1[:], outT[kc][:, il], wo[0][:, kc, 512:1024],
                             start=(kc == 0), stop=(kc == KC - 1))
        f0 = pf.tile([128, 512], F32, tag="fout")
        nc.vector.tensor_add(f0[:], ps0[:], bo_bc[:, 0:512])
        nc.gpsimd.dma_start(out=out[il, 0:512], in_=f0[:])
        f1 = pf.tile([128, 512], F32, tag="fout")
        nc.vector.tensor_add(f1[:], ps1[:], bo_bc[:, 512:1024])
        nc.gpsimd.dma_start(out=out[il, 512:1024], in_=f1[:])

    def unit(p, ic, kp):
        """Attention for head pair p, query chunk ic (512 rows)."""
        hA, hB = 2 * p, 2 * p + 1
        isl = slice(ic * 512, (ic + 1) * 512)
        # rows 0-63: head output; row 64: Z (from vpa's ones column)
        ozA = psO.tile([65, 512], F32, tag="ozA")
        ozB = psO.tile([65, 512], F32, tag="ozB")
        for j in range(NJ):
            jsl = slice(j * 128, (j + 1) * 128)
            s = psS.tile([128, 1024], F32, tag="s")
            nc.tensor.matmul(s[:, 0:512], kp[0:64, jsl], qpT[p][0:64, isl],
                             tile_position=(0, 0))
            nc.tensor.matmul(s[:, 512:1024], kp[64:128, jsl],
                             qpT[p][64:128, isl], tile_position=(64, 0))
            e = pe_.tile([128, 1024], BF16, tag="e")
            nc.scalar.activation(e[:], s[:], mybir.ActivationFunctionType.Exp)
            if dbg and p == 0 and ic == 0 and j == 0:
                nc.gpsimd.dma_start(out=d_e[:], in_=e[:])
            nc.tensor.matmul(ozA[:], vpa[j][:, hA * 65:(hA + 1) * 65],
                             e[:, 0:512], start=(j == 0), stop=(j == NJ - 1))
            nc.tensor.matmul(ozB[:], vpa[j][:, hB * 65:(hB + 1) * 65],
                             e[:, 512:1024], start=(j == 0), stop=(j == NJ - 1))
        if dbg and p == 0 and ic == 0:
            ozc = pz.tile([65, 1024], F32, tag="zb")
            nc.vector.tensor_copy(ozc[:, 0:512], ozA[:])
            nc.vector.tensor_copy(ozc[:, 512:1024], ozB[:])
            nc.sync.dma_start(out=d_oz[0:65, :], in_=ozc[:, 0:512])
            nc.sync.dma_start(out=d_oz[65:130, :], in_=ozc[:, 512:1024])
        # Z rows live at psum partition 64. Broadcast Z to partitions 0:64
        # via a K=1 matmul (reads partition 64), then take 1/Z on the
        # resulting base-0 SBUF tile.
        zb = pz.tile([65, 1024], BF16, tag="zb")
        nc.vector.tensor_copy(zb[64:65, 0:512], ozA[64:65, :])
        nc.vector.tensor_copy(zb[64:65, 512:1024], ozB[64:65, :])
        rA = psX.tile([64, 512], F32, tag="aux")
        nc.tensor.matmul(rA[:], cones[64:65, 0:64], zb[64:65, 0:512])
        rsbA = pz.tile([64, 512], F32, tag="rsbA")
        nc.vector.tensor_copy(rsbA[:], rA[:])
        riA = pz.tile([64, 512], F32, tag="riA")
        nc.vector.reciprocal_approx_fast(riA[:], rsbA[:])
        if dbg and p == 0 and ic == 0:
            nc.sync.dma_start(out=d_z[0:1, :], in_=riA[0:1, :])
        nc.vector.tensor_mul(outT[p][0:64, isl], ozA[0:64, :], riA[:])
        rB = psX.tile([64, 512], F32, tag="aux")
        nc.tensor.matmul(rB[:], cones[64:65, 0:64], zb[64:65, 512:1024])
        rsbB = pz.tile([64, 512], F32, tag="rsbB")
        nc.vector.tensor_copy(rsbB[:], rB[:])
        riB = pz.tile([64, 512], F32, tag="riB")
        nc.vector.reciprocal_approx_fast(riB[:], rsbB[:])
        stB = pz.tile([64, 512], BF16, tag="stB")
        nc.vector.tensor_mul(stB[:], ozB[0:64, :], riB[:])
        # head B's rows live at partitions 0:64; shift to outT rows 64:128
        nc.gpsimd.dma_start(out=outT[p][64:128, isl], in_=stB[:])

    # ---- emission schedule (p-major, fillers keep the PE dense) ---------
    for c in range(4):
        vp_block(0, c)
    qpT_all()
    kp_tiles = {0: kpT_block(0)}

    def load_wo():
        wo_t = pwvo.tile([128, KC, D], BF16, tag="wvo")
        nc.sync.dma_start(out=wo_t[:], in_=woT_r[:])
        wo.append(wo_t)

    fillers = {
        1: [lambda: kp_tiles.__setitem__(1, kpT_block(1))],
        2: [lambda: kp_tiles.__setitem__(2, kpT_block(2)),
            lambda: vp_block(1, 0), lambda: vp_block(1, 1)],
        3: [lambda: kp_tiles.__setitem__(3, kpT_block(3)),
            lambda: vp_block(1, 2), lambda: vp_block(1, 3)],
        4: [lambda: kp_tiles.__setitem__(4, kpT_block(4)), load_wo],
        5: [lambda: kp_tiles.__setitem__(5, kpT_block(5))],
        6: [lambda: kp_tiles.__setitem__(6, kpT_block(6))],
        7: [lambda: kp_tiles.__setitem__(7, kpT_block(7))],
    }

    for p in range(NPAIR):
        for fn in fillers.get(p, []):
            fn()
        if dbg and p == 0:
            nc.sync.dma_start(out=d_kp[:], in_=kp_tiles[0][:])
            nc.sync.dma_start(out=d_qp[:], in_=qpT[0][:])
            nc.sync.dma_start(out=d_vpa[:], in_=vpa[0][:])
        unit(p, 0, kp_tiles[p])
        if p == NPAIR - 1:
            # outT cols 0:512 (ic=0) complete: fill unit(7,1)'s ACT-bound
            # window with the first half of the out-projection (out row
            # blocks i<4 contract only outT[:, 0:512])
            for i in range(4):
                outproj_block(i)
        unit(p, 1, kp_tiles[p])
    if dbg:
        nc.sync.dma_start(out=d_outT[:], in_=outT[0][:])

    for i in range(4, KC):
        outproj_block(i)

    top.close()


_NC_CACHE = None


def _get_program():
    global _NC_CACHE
    if _NC_CACHE is None:
        _NC_CACHE = build_program()
    return _NC_CACHE


def prep_in_maps(q, k, v, w_q, b_q, w_k, b_k, w_v, b_v, w_o, b_o):
    import ml_dtypes

    f = np.float32
    bf = ml_dtypes.bfloat16
    q, k, v = (np.asarray(t, f) for t in (q, k, v))
    scale = 1.0 / np.sqrt(DH)
    wqT = np.ascontiguousarray((np.asarray(w_q, f) * scale).T.astype(bf))
    wkT = np.ascontiguousarray(np.asarray(w_k, f).T.astype(bf))
    wvT = np.ascontiguousarray(np.asarray(w_v, f).T.astype(bf))
    woT = np.ascontiguousarray(np.asarray(w_o, f).T.astype(bf))
    bqc = np.ascontiguousarray((np.asarray(b_q, f) * scale).reshape(KC, 128).T)
    bkc = np.ascontiguousarray(np.asarray(b_k, f).reshape(KC, 128).T)
    bvr = np.asarray(b_v, f).reshape(1, D).astype(bf)
    bor = np.asarray(b_o, f).reshape(1, D).astype(bf)
    in_maps = []
    for c in range(N_CORES):
        b, qh = c // 2, c % 2
        kTb = np.ascontiguousarray(k[b].T.astype(bf))
        vTb = np.ascontiguousarray(v[b].T.astype(bf))
        qTb = np.ascontiguousarray(q[b].T[:, qh * LQ:(qh + 1) * LQ].astype(bf))
        in_maps.append({
            "xqT": qTb, "xkT": kTb, "xvT": vTb,
            "wqT": wqT, "wkT": wkT, "wvT": wvT, "woT": woT,
            "bqc": bqc, "bkc": bkc, "bvr": bvr, "bor": bor,
        })
    return in_maps


def run(in_maps, trace=False, **kw):
    nc = _get_program()
    return run_bass_kernel_spmd(nc, in_maps, list(range(N_CORES)),
                                trace=trace, **kw)


def kernel(**inputs):
    in_maps = prep_in_maps(**inputs)
    res = run(in_maps)
    out = np.empty((B, L, D), np.float32)
    for c in range(N_CORES):
        b, qh = c // 2, c % 2
        out[b, qh * LQ:(qh + 1) * LQ, :] = res.results[c]["out"]
    return out
